# revision 60
# baseline (speedup 1.0000x reference)
"""Trainium2 Bass kernel for single-head causal attention.

Problem: x:[4,2048,768], Wq/Wk/Wv:[768,768] (torch-Linear layout, y = x @ W.T),
out = causal_softmax(q k^T / sqrt(768)) @ v, all float32.

Sharding (8 NeuronCores, no collectives):
  - core pair (2b, 2b+1) handles batch b.
  - per batch, the 16 query tiles of 128 rows are split between the pair as
    {0,3,4,7,8,11,12,15} and {1,2,5,6,9,10,13,14}. Sorted by causal length
    those are {1,4,5,8,9,12,13,16} and {2,3,6,7,10,11,14,15} key-tiles, so
    both sides fit the same static per-slot key budget {2,4,...,16}: the one
    SPMD graph processes 8 query tiles whose key ranges are padded by at most
    one 128-tile (+6% flops) and the pad/diagonal is handled by a host-
    provided additive mask over the last two key blocks of every slot.
  - scores are computed as (x_q @ M) @ x_k^T with M = Wq^T @ Wk folded on the
    host, so the device never projects K: the "g" projection x_q @ M replaces
    the Q projection at identical cost and the raw transposed x serves as the
    key operand directly.
  - scores are computed TRANSPOSED (key-major): scoresT[k, q] tiles come out
    of the PE in exactly the layout the probs @ x matmul needs as stationary
    operand, so the 128x128 probability transposes disappear. The softmax
    denominator is obtained by appending a ones-column to the x rows used in
    the AV matmul (out_ps[:, 768] = sum_k p_k).
  - host pre-transposes inputs (x^T, M, W^T), packs them into the SBUF chunk
    layout [128, chunk, width], and converts to bf16, so the device does no
    operand transposes; only the per-query-tile Y = probs @ x results are
    transposed on the TensorEngine to apply Wv.
"""

import math
import os
import sys

import numpy as np

if not any(os.path.isdir(os.path.join(p, "concourse")) for p in sys.path):
    sys.path.insert(0, "/opt/trn_rl_repo")

import concourse.bass as bass  # noqa: E402
import concourse.mybir as mybir  # noqa: E402
from concourse import bacc, tile  # noqa: E402
from concourse.bass_utils import run_bass_kernel_spmd  # noqa: E402
from concourse.masks import make_identity  # noqa: E402

import ml_dtypes  # noqa: E402

B, S, D = 4, 2048, 768
P = 128
NT = S // P          # 16 key tiles per batch
DC = D // P          # 6 contraction chunks
NSLOT = 8            # query tiles per core
QROWS = NSLOT * P    # 1024 query rows per core
N_CORES = 8
SCALE = 1.0 / math.sqrt(D)

SIDE_A = [0, 3, 4, 7, 8, 11, 12, 15]   # causal lengths 1,4,5,8,9,12,13,16
SIDE_B = [1, 2, 5, 6, 9, 10, 13, 14]   # causal lengths 2,3,6,7,10,11,14,15
CAP = [2, 4, 6, 8, 10, 12, 14, 16]     # static key tiles per slot (>= real)
DV = D + 4                             # x rows padded with a ones column

BF16 = ml_dtypes.bfloat16

_NC = None


def build():
    """Build + compile the single SPMD graph run by all 8 cores."""
    f32 = mybir.dt.float32
    bf16 = mybir.dt.bfloat16

    nc = bacc.Bacc("TRN2", target_bir_lowering=False, debug=False,
                   num_devices=N_CORES)

    # inputs come pre-packed as [P, chunk, width] (host layout transform)
    xq_d = nc.dram_tensor("xqT", [P, 2, DC, 512], bf16,
                          kind="ExternalInput").ap()
    xkv_d = nc.dram_tensor("xkvT", [P, 2, DC, S // 2], bf16,
                           kind="ExternalInput").ap()
    xkvr_d = nc.dram_tensor("xkvR", [P, NT, DV], bf16,
                            kind="ExternalInput").ap()
    wq_d = nc.dram_tensor("mT", [P, DC, D], bf16,
                          kind="ExternalInput").ap()
    wv_d = nc.dram_tensor("wvT", [P, DC, D], bf16, kind="ExternalInput").ap()
    mask_d = nc.dram_tensor("mask", [P, NT, P], bf16,
                            kind="ExternalInput").ap()
    out_d = nc.dram_tensor("out", [QROWS, D], f32, kind="ExternalOutput").ap()

    with tile.TileContext(nc) as tc:
        with (
            tc.tile_pool(name="const", bufs=1) as const,
            tc.tile_pool(name="pt", bufs=3) as pt_pool,
            tc.tile_pool(name="osb", bufs=2) as osb_pool,
            tc.tile_pool(name="small", bufs=2) as small,
            tc.tile_pool(name="ps_s", bufs=2, space="PSUM") as ps_s,
            tc.tile_pool(name="ps_tr", bufs=2, space="PSUM") as ps_tr,
            tc.tile_pool(name="ps_o", bufs=2, space="PSUM") as ps_o,
        ):
            # ---- persistent SBUF tensors, split in two halves of 3 d-chunks
            # each so input DMA (12KB+ descriptors) overlaps the projections
            HC = DC // 2
            wq_h = [const.tile([P, HC, D], bf16, tag=f"wqh{h}", name=f"wqh{h}")
                    for h in range(2)]
            wv_h = [const.tile([P, HC, D], bf16, tag=f"wvh{h}", name=f"wvh{h}")
                    for h in range(2)]
            # group 0 in two half-tiles so its first matmuls only wait on the
            # first half's DMA; group 1 arrives while group 0 is computed
            xq_0 = [const.tile([P, HC, 512], bf16, tag=f"xq0{h}",
                               name=f"xq0{h}") for h in range(2)]
            xq_1 = const.tile([P, DC, 512], bf16, tag="xq1")

            def xq_chunk(g, dc):
                if g == 0:
                    return xq_0[dc // HC][:, dc % HC, :]
                return xq_1[:, dc, :]
            # keys split by k-range so scores can start after half the DMA
            xkv_k = [const.tile([P, DC, S // 2], bf16, tag=f"xkvk{h}",
                                name=f"xkvk{h}") for h in range(2)]

            def chunk(tiles, dc):
                return tiles[dc // HC][:, dc % HC, :]

            wv_c = [chunk(wv_h, c) for c in range(DC)]

            def xkv_c(dc, kt):
                # [128 d, 128 k] block of x^T for key tile kt, d-chunk dc
                j, kk = divmod(kt, NT // 2)
                return xkv_k[j][:, dc, kk * P:(kk + 1) * P]
            mask_sb = const.tile([P, NT, P], bf16, tag="mask")
            ident = const.tile([P, P], bf16, tag="ident")
            qt_sb = const.tile([P, DC, QROWS], bf16, tag="qt")
            xv_q = [const.tile([P, NT // 4, DV], bf16, tag=f"xvq{h}",
                               name=f"xvq{h}") for h in range(4)]
            # persistent transposed-probability tiles, one per key tile kt:
            # [128 k, (NSLOT - kt//2)*128 q] covering slots kt//2 .. 7
            pT_k = [const.tile([P, (NSLOT - kt // 2) * P], bf16,
                               tag=f"pTk{kt}", name=f"pTk{kt}")
                    for kt in range(NT)]

            # priority-ordered input DMAs, descriptor issue split between the
            # two HWDGE engines (Sync: x tensors, Scalar: weights/mask) so
            # posting isn't serialized on Sync at ~0.6us per dma_start.
            # xq group 0 and M stream in d-chunk halves so the projection can
            # begin accumulating as soon as the first halves land.
            nc.sync.dma_start(out=xq_0[0][:, :, :], in_=xq_d[:, 0, 0:HC, :])
            nc.scalar.dma_start(out=wq_h[0][:, :, :], in_=wq_d[:, 0:HC, :])
            nc.sync.dma_start(out=xq_0[1][:, :, :], in_=xq_d[:, 0, HC:DC, :])
            nc.scalar.dma_start(out=wq_h[1][:, :, :], in_=wq_d[:, HC:DC, :])
            make_identity(nc, ident[:, :])

            # HAM warm-up: keep the PE busy (and the clock ramping) while the
            # first input halves stream in
            warm = ps_tr.tile([P, P], f32, tag="tr", name="warm")
            for _ in range(52):
                nc.tensor.matmul(warm[:, :], ident[:, :], ident[:, :],
                                 start=True, stop=True)

            # ---- gT[dout,q] = (x_q @ M)^T projection (group-major, streamed
            # under the input DMA; M = Wq^T Wk folded on host)
            def emit_gproj(g):
                for oc in range(DC):
                    ps = ps_s.tile([P, 512], f32, tag="mm512")
                    for dc in range(DC):
                        if g == 0 and oc == 0 and dc == HC:
                            # first visit to the second input halves: fill the
                            # PE with more warm-up while their DMA completes
                            for _ in range(52):
                                nc.tensor.matmul(warm[:, :], ident[:, :],
                                                 ident[:, :],
                                                 start=True, stop=True)
                        nc.tensor.matmul(
                            ps[:, :],
                            wq_h[dc // HC][:, dc % HC,
                                           oc * P:(oc + 1) * P],
                            xq_chunk(g, dc),
                            start=(dc == 0), stop=(dc == DC - 1))
                    nc.scalar.copy(qt_sb[:, oc, g * 512:(g + 1) * 512],
                                   ps[:, :])

            emit_gproj(0)

            # stream the attention operands while gT group 0 is computed;
            # first-use order: first-half keys + mask for the slot 0-3 chain,
            # its x rows, then group-1 queries and the second-half keys
            qn = NT // 4
            nc.sync.dma_start(out=xkv_k[0][:, :, :], in_=xkv_d[:, 0, :, :])
            nc.scalar.dma_start(out=mask_sb[:, :, :], in_=mask_d[:, :, :])
            nc.sync.dma_start(out=xv_q[0][:, :, :], in_=xkvr_d[:, 0:qn, :])
            nc.scalar.dma_start(out=wv_h[0][:, :, :], in_=wv_d[:, 0:HC, :])
            nc.sync.dma_start(out=xv_q[1][:, :, :], in_=xkvr_d[:, qn:2 * qn, :])
            nc.sync.dma_start(out=xq_1[:, :, :], in_=xq_d[:, 1, :, :])
            nc.sync.dma_start(out=xkv_k[1][:, :, :], in_=xkv_d[:, 1, :, :])
            nc.scalar.dma_start(out=wv_h[1][:, :, :], in_=wv_d[:, HC:DC, :])
            for qtr in range(2, 4):
                nc.sync.dma_start(out=xv_q[qtr][:, :, :],
                                  in_=xkvr_d[:, qtr * qn:(qtr + 1) * qn, :])

            # ---- attention: key-major transposed scores, slot-major AV.
            # score tiles are emitted in pieces split at the q=512 group
            # boundary, so part 0 (query slots 0-3) only depends on the first
            # half of the gT projection and can run before group 1 exists.
            def emit_sc(kt, part):
                s0 = kt // 2                    # first (and only masked) slot
                pT = pT_k[kt]
                if kt < NT // 2 and part == 0:
                    c0, c1 = s0 * P, 512        # group-0 columns, has mask
                    masked = True
                elif kt < NT // 2:
                    c0, c1 = 512, QROWS         # group-1 columns
                    masked = False
                else:
                    c0, c1 = s0 * P, QROWS      # entirely inside group 1
                    masked = True
                pw = c1 - c0
                ps = ps_s.tile([P, 512], f32, tag="mm512",
                               name=f"tps{kt}_{part}")
                for dc in range(DC):
                    nc.tensor.matmul(
                        ps[:, :pw], xkv_c(dc, kt), qt_sb[:, dc, c0:c1],
                        start=(dc == 0), stop=(dc == DC - 1))
                if masked:                      # diagonal/pad mask: slot s0
                    nc.vector.tensor_add(ps[:, 0:P], ps[:, 0:P],
                                         mask_sb[:, kt, :])
                nc.scalar.activation(
                    pT[:, c0 - s0 * P:c1 - s0 * P], ps[:, :pw],
                    mybir.ActivationFunctionType.Exp, scale=SCALE)

            slot_bufs = {}

            def emit_av(s):
                L = CAP[s]
                out_ps = ps_o.tile([P, DV], f32, tag="mmout",
                                   name=f"ops{s}")
                for kt in range(L):
                    lhs = pT_k[kt][:, (s - kt // 2) * P:(s - kt // 2 + 1) * P]
                    xv = xv_q[kt // 4][:, kt % 4, :]
                    nc.tensor.matmul(out_ps[:, 0:512], lhs, xv[:, 0:512],
                                     start=(kt == 0), stop=(kt == L - 1))
                    nc.tensor.matmul(out_ps[:, 512:DV], lhs, xv[:, 512:DV],
                                     start=(kt == 0), stop=(kt == L - 1))
                rinv = small.tile([P, 1], f32, tag="rinv", name=f"rinv{s}")
                nc.vector.reciprocal(rinv[:, :], out_ps[:, D:D + 1])
                # out_ps holds Y = probs @ x_kv [q, d]; stage Y/l to SBUF
                # (softmax normalization folded into the drain copies) so it
                # can be transposed to apply Wv: out = (Y/l) @ Wv^T
                y_lo = osb_pool.tile([P, 512], bf16, tag="ylo", name=f"ylo{s}")
                y_hi = osb_pool.tile([P, 256], bf16, tag="yhi", name=f"yhi{s}")
                nc.scalar.activation(y_lo[:, :], out_ps[:, 0:512],
                                     mybir.ActivationFunctionType.Copy,
                                     scale=rinv[:, :])
                nc.vector.tensor_scalar_mul(y_hi[:, :], out_ps[:, 512:D],
                                            rinv[:, :])
                slot_bufs[s] = (y_lo, y_hi)

            def emit_ytr(s):
                y_lo, y_hi = slot_bufs.pop(s)
                ytT = pt_pool.tile([P, D], bf16, tag="ytT", name=f"ytT{s}")
                for kg in range(2):
                    kn = 4 if kg == 0 else 2
                    tp = ps_tr.tile([P, 512], bf16, tag="tr", name=f"ytp{s}")
                    for j in range(kn):
                        dt = kg * 4 + j
                        ysrc = (y_lo[:, dt * P:(dt + 1) * P] if dt < 4 else
                                y_hi[:, (dt - 4) * P:(dt - 3) * P])
                        nc.tensor.transpose(tp[:, j * P:(j + 1) * P],
                                            ysrc, ident[:, :])
                    nc.vector.tensor_copy(
                        ytT[:, kg * 512:kg * 512 + kn * P],
                        tp[:, 0:kn * P])
                slot_bufs[s] = ytT

            def emit_out2(s):
                ytT = slot_bufs.pop(s)
                last = s == NSLOT - 1
                out2_ps = ps_o.tile([P, D], f32, tag="mmout", name=f"o2ps{s}")
                out_sb = osb_pool.tile([P, D], f32, tag="osb", name=f"osb{s}")

                def drain(c0, c1, eng):
                    cp = (nc.scalar.copy if eng == 0 else
                          nc.vector.tensor_copy)
                    cp(out_sb[:, c0:c1], out2_ps[:, c0:c1])
                    nc.sync.dma_start(out=out_d[s * P:(s + 1) * P, c0:c1],
                                      in_=out_sb[:, c0:c1])

                if not last:
                    for dc in range(DC):
                        nc.tensor.matmul(out2_ps[:, 0:512],
                                         ytT[:, dc * P:(dc + 1) * P],
                                         wv_c[dc][:, 0:512],
                                         start=(dc == 0), stop=(dc == DC - 1))
                    for dc in range(DC):
                        nc.tensor.matmul(out2_ps[:, 512:D],
                                         ytT[:, dc * P:(dc + 1) * P],
                                         wv_c[dc][:, 512:D],
                                         start=(dc == 0), stop=(dc == DC - 1))
                    # already normalized; PSUM->SBUF drain split across the
                    # scalar and vector engines, halves DMA'd independently
                    drain(0, 384, 0)
                    drain(384, D, 1)
                else:
                    # last slot: three accumulation groups drained in narrow
                    # strips so the kernel tail is one short copy + DMA
                    for g, (c0, c1) in enumerate([(0, 512), (512, 640),
                                                  (640, 768)]):
                        ps_g = (out2_ps if g == 0 else
                                ps_s.tile([P, 512], f32, tag="mm512",
                                          name=f"o2t{s}_{g}"))
                        o0 = 0 if g == 0 else c0
                        for dc in range(DC):
                            nc.tensor.matmul(ps_g[:, c0 - o0:c1 - o0],
                                             ytT[:, dc * P:(dc + 1) * P],
                                             wv_c[dc][:, c0:c1],
                                             start=(dc == 0),
                                             stop=(dc == DC - 1))
                        if g == 0:
                            nc.scalar.copy(out_sb[:, 0:256], out2_ps[:, 0:256])
                            nc.scalar.dma_start(
                                out=out_d[s * P:(s + 1) * P, 0:256],
                                in_=out_sb[:, 0:256])
                            nc.vector.tensor_copy(out_sb[:, 256:512],
                                                  out2_ps[:, 256:512])
                            nc.sync.dma_start(
                                out=out_d[s * P:(s + 1) * P, 256:512],
                                in_=out_sb[:, 256:512])
                        else:
                            cp = (nc.scalar.copy if g == 1 else
                                  nc.vector.tensor_copy)
                            cp(out_sb[:, c0:c1], ps_g[:, 0:c1 - c0])
                            dma = (nc.scalar.dma_start if g == 1 else
                                   nc.sync.dma_start)
                            dma(out=out_d[s * P:(s + 1) * P, c0:c1],
                                in_=out_sb[:, c0:c1])

            # group-decoupled pipeline: the part-0 score pieces and the whole
            # slot 0-3 chain depend only on gT group 0, so they run while the
            # group-1 inputs stream in; gproj group 1 is emitted mid-body.
            for kt in range(NT // 2):
                emit_sc(kt, 0)                  # slots 0-3 scorelets
            emit_av(0)
            emit_av(1)
            emit_ytr(0)
            emit_av(2)
            emit_ytr(1)
            emit_out2(0)
            emit_av(3)
            emit_ytr(2)
            emit_out2(1)
            emit_gproj(1)                       # q-group 1 arrives during the
            for kt in range(NT // 2):           # slot 0-3 tail
                emit_sc(kt, 1)
                if kt == 2:
                    emit_ytr(3)
                    emit_out2(2)
            emit_sc(8, 1)
            emit_sc(9, 1)
            emit_av(4)
            emit_out2(3)
            for s in range(5, NSLOT):
                emit_sc(2 * s, 1)
                emit_sc(2 * s + 1, 1)
                emit_ytr(s - 1)
                emit_av(s)
                emit_out2(s - 1)
            emit_ytr(NSLOT - 1)
            emit_out2(NSLOT - 1)

    nc.compile()
    return nc


def _pack(matT):
    """[D, W] (transposed operand) -> [P, DC, W] chunk layout, bf16."""
    d, w = matT.shape
    return np.ascontiguousarray(
        matT.reshape(d // P, P, w).transpose(1, 0, 2)).astype(BF16)


def shard_inputs(x, Wq, Wk, Wv):
    x = np.asarray(x, dtype=np.float32)
    # scores = (x_q Wq^T)(x_k Wk^T)^T = x_q (Wq^T Wk) x_k^T: fold the two
    # projection matrices into M on the host; the device projects only x_q
    M = np.asarray(Wq, np.float32).T @ np.asarray(Wk, np.float32)
    mT = _pack(M)                                        # [P, DC(din), dout]
    wvT = _pack(np.asarray(Wv, np.float32).T)
    in_maps = []
    for c in range(N_CORES):
        b, side = divmod(c, 2)
        qtiles = SIDE_A if side == 0 else SIDE_B
        xb = x[b]                                    # [S, D]
        xkvT = _pack(np.ascontiguousarray(xb.T))     # [P, DC, S]
        xkvT = np.ascontiguousarray(                 # [P, 2, DC, S//2]
            xkvT.reshape(P, DC, 2, S // 2).transpose(0, 2, 1, 3))
        xkvR = np.zeros((P, NT, DV), BF16)           # row-major + ones column
        xkvR[:, :, :D] = xb.astype(BF16).reshape(NT, P, D).transpose(1, 0, 2)
        xkvR[:, :, D] = 1.0
        xq = np.concatenate([xb[t * P:(t + 1) * P] for t in qtiles], axis=0)
        xqT = _pack(np.ascontiguousarray(xq.T))          # [P, DC, QROWS]
        xqT = np.ascontiguousarray(                      # [P, 2, DC, 512]
            xqT.reshape(P, DC, 2, 512).transpose(0, 2, 1, 3))
        # transposed mask per key tile kt: [128 k, 128 q] for slot kt//2
        mask = np.empty((P, NT, P), np.float32)
        for kt in range(NT):
            t0 = qtiles[kt // 2]
            kidx = kt * P + np.arange(P)[:, None]
            qidx = t0 * P + np.arange(P)[None, :]
            mask[:, kt, :] = np.where(kidx <= qidx, 0.0, -1e30)
        mask = mask.astype(BF16)
        in_maps.append({"xqT": xqT, "xkvT": xkvT, "xkvR": xkvR, "mT": mT,
                        "wvT": wvT, "mask": mask})
    return in_maps


def unshard(results):
    out = np.empty((B, S, D), np.float32)
    for c in range(N_CORES):
        b, side = divmod(c, 2)
        qtiles = SIDE_A if side == 0 else SIDE_B
        oc = results[c]["out"]
        for s, t in enumerate(qtiles):
            out[b, t * P:(t + 1) * P] = oc[s * P:(s + 1) * P]
    return out


def run(inputs, trace=False, trace_cores=None):
    """Run on hardware; returns (output, BassKernelResults)."""
    global _NC
    if _NC is None:
        _NC = build()
    in_maps = shard_inputs(inputs["x"], inputs["Wq"], inputs["Wk"],
                           inputs["Wv"])
    res = run_bass_kernel_spmd(_NC, in_maps, core_ids=list(range(N_CORES)),
                               trace=trace, trace_cores=trace_cores)
    return unshard(res.results), res


def kernel(x, Wq, Wk, Wv):
    out, _ = run({"x": x, "Wq": Wq, "Wk": Wk, "Wv": Wv})
    return out



# revision 62
# speedup vs baseline: 1.0021x; 1.0021x over previous
"""Trainium2 Bass kernel for single-head causal attention.

Problem: x:[4,2048,768], Wq/Wk/Wv:[768,768] (torch-Linear layout, y = x @ W.T),
out = causal_softmax(q k^T / sqrt(768)) @ v, all float32.

Sharding (8 NeuronCores, no collectives):
  - core pair (2b, 2b+1) handles batch b.
  - per batch, the 16 query tiles of 128 rows are split between the pair as
    {0,3,4,7,8,11,12,15} and {1,2,5,6,9,10,13,14}. Sorted by causal length
    those are {1,4,5,8,9,12,13,16} and {2,3,6,7,10,11,14,15} key-tiles, so
    both sides fit the same static per-slot key budget {2,4,...,16}: the one
    SPMD graph processes 8 query tiles whose key ranges are padded by at most
    one 128-tile (+6% flops) and the pad/diagonal is handled by a host-
    provided additive mask over the last two key blocks of every slot.
  - scores are computed as (x_q @ M) @ x_k^T with M = Wq^T @ Wk folded on the
    host, so the device never projects K: the "g" projection x_q @ M replaces
    the Q projection at identical cost and the raw transposed x serves as the
    key operand directly.
  - scores are computed TRANSPOSED (key-major): scoresT[k, q] tiles come out
    of the PE in exactly the layout the probs @ x matmul needs as stationary
    operand, so the 128x128 probability transposes disappear. The softmax
    denominator is obtained by appending a ones-column to the x rows used in
    the AV matmul (out_ps[:, 768] = sum_k p_k).
  - host pre-transposes inputs (x^T, M, W^T), packs them into the SBUF chunk
    layout [128, chunk, width], and converts to bf16, so the device does no
    operand transposes; only the per-query-tile Y = probs @ x results are
    transposed on the TensorEngine to apply Wv.
"""

import math
import os
import sys

import numpy as np

if not any(os.path.isdir(os.path.join(p, "concourse")) for p in sys.path):
    sys.path.insert(0, "/opt/trn_rl_repo")

import concourse.bass as bass  # noqa: E402
import concourse.mybir as mybir  # noqa: E402
from concourse import bacc, tile  # noqa: E402
from concourse.bass_utils import run_bass_kernel_spmd  # noqa: E402
from concourse.masks import make_identity  # noqa: E402

import ml_dtypes  # noqa: E402

B, S, D = 4, 2048, 768
P = 128
NT = S // P          # 16 key tiles per batch
DC = D // P          # 6 contraction chunks
NSLOT = 8            # query tiles per core
QROWS = NSLOT * P    # 1024 query rows per core
N_CORES = 8
SCALE = 1.0 / math.sqrt(D)

SIDE_A = [0, 3, 4, 7, 8, 11, 12, 15]   # causal lengths 1,4,5,8,9,12,13,16
SIDE_B = [1, 2, 5, 6, 9, 10, 13, 14]   # causal lengths 2,3,6,7,10,11,14,15
CAP = [2, 4, 6, 8, 10, 12, 14, 16]     # static key tiles per slot (>= real)
DV = D + 4                             # x rows padded with a ones column

BF16 = ml_dtypes.bfloat16

_NC = None


def build():
    """Build + compile the single SPMD graph run by all 8 cores."""
    f32 = mybir.dt.float32
    bf16 = mybir.dt.bfloat16

    nc = bacc.Bacc("TRN2", target_bir_lowering=False, debug=False,
                   num_devices=N_CORES)

    # inputs come pre-packed as [P, chunk, width] (host layout transform)
    xq_d = nc.dram_tensor("xqT", [P, 2, DC, 512], bf16,
                          kind="ExternalInput").ap()
    xkv_d = nc.dram_tensor("xkvT", [P, 2, DC, S // 2], bf16,
                           kind="ExternalInput").ap()
    xkvr_d = nc.dram_tensor("xkvR", [P, NT, DV], bf16,
                            kind="ExternalInput").ap()
    wq_d = nc.dram_tensor("mT", [P, DC, D], bf16,
                          kind="ExternalInput").ap()
    wv_d = nc.dram_tensor("wvT", [P, DC, D], bf16, kind="ExternalInput").ap()
    mask_d = nc.dram_tensor("mask", [P, NT, P], bf16,
                            kind="ExternalInput").ap()
    out_d = nc.dram_tensor("out", [QROWS, D], f32, kind="ExternalOutput").ap()

    with tile.TileContext(nc) as tc:
        with (
            tc.tile_pool(name="const", bufs=1) as const,
            tc.tile_pool(name="pt", bufs=3) as pt_pool,
            tc.tile_pool(name="osb", bufs=2) as osb_pool,
            tc.tile_pool(name="small", bufs=2) as small,
            tc.tile_pool(name="ps_s", bufs=2, space="PSUM") as ps_s,
            tc.tile_pool(name="ps_tr", bufs=2, space="PSUM") as ps_tr,
            tc.tile_pool(name="ps_o", bufs=2, space="PSUM") as ps_o,
        ):
            # ---- persistent SBUF tensors, split in two halves of 3 d-chunks
            # each so input DMA (12KB+ descriptors) overlaps the projections
            HC = DC // 2
            wq_h = [const.tile([P, HC, D], bf16, tag=f"wqh{h}", name=f"wqh{h}")
                    for h in range(2)]
            wv_h = [const.tile([P, HC, D], bf16, tag=f"wvh{h}", name=f"wvh{h}")
                    for h in range(2)]
            # group 0 in two half-tiles so its first matmuls only wait on the
            # first half's DMA; group 1 arrives while group 0 is computed
            xq_0 = [const.tile([P, HC, 512], bf16, tag=f"xq0{h}",
                               name=f"xq0{h}") for h in range(2)]
            xq_1 = const.tile([P, DC, 512], bf16, tag="xq1")

            def xq_chunk(g, dc):
                if g == 0:
                    return xq_0[dc // HC][:, dc % HC, :]
                return xq_1[:, dc, :]
            # keys split by k-range so scores can start after half the DMA
            xkv_k = [const.tile([P, DC, S // 2], bf16, tag=f"xkvk{h}",
                                name=f"xkvk{h}") for h in range(2)]

            def chunk(tiles, dc):
                return tiles[dc // HC][:, dc % HC, :]

            wv_c = [chunk(wv_h, c) for c in range(DC)]

            def xkv_c(dc, kt):
                # [128 d, 128 k] block of x^T for key tile kt, d-chunk dc
                j, kk = divmod(kt, NT // 2)
                return xkv_k[j][:, dc, kk * P:(kk + 1) * P]
            mask_sb = const.tile([P, NT, P], bf16, tag="mask")
            ident = const.tile([P, P], bf16, tag="ident")
            qt_sb = const.tile([P, DC, QROWS], bf16, tag="qt")
            xv_q = [const.tile([P, NT // 4, DV], bf16, tag=f"xvq{h}",
                               name=f"xvq{h}") for h in range(4)]
            # persistent transposed-probability tiles, one per key tile kt:
            # [128 k, (NSLOT - kt//2)*128 q] covering slots kt//2 .. 7
            pT_k = [const.tile([P, (NSLOT - kt // 2) * P], bf16,
                               tag=f"pTk{kt}", name=f"pTk{kt}")
                    for kt in range(NT)]

            # priority-ordered input DMAs, descriptor issue split between the
            # two HWDGE engines (Sync: x tensors, Scalar: weights/mask) so
            # posting isn't serialized on Sync at ~0.6us per dma_start.
            # xq group 0 and M stream in d-chunk halves so the projection can
            # begin accumulating as soon as the first halves land.
            nc.sync.dma_start(out=xq_0[0][:, :, :], in_=xq_d[:, 0, 0:HC, :])
            nc.scalar.dma_start(out=wq_h[0][:, :, :], in_=wq_d[:, 0:HC, :])
            nc.sync.dma_start(out=xq_0[1][:, :, :], in_=xq_d[:, 0, HC:DC, :])
            nc.scalar.dma_start(out=wq_h[1][:, :, :], in_=wq_d[:, HC:DC, :])
            make_identity(nc, ident[:, :])

            # HAM warm-up: keep the PE busy (and the clock ramping) while the
            # first input halves stream in
            warm = ps_tr.tile([P, P], f32, tag="tr", name="warm")
            for _ in range(36):
                nc.tensor.matmul(warm[:, :], ident[:, :], ident[:, :],
                                 start=True, stop=True)

            # ---- gT[dout,q] = (x_q @ M)^T projection (group-major, streamed
            # under the input DMA; M = Wq^T Wk folded on host)
            def emit_gproj(g):
                for oc in range(DC):
                    ps = ps_s.tile([P, 512], f32, tag="mm512")
                    for dc in range(DC):
                        if g == 0 and oc == 0 and dc == HC:
                            # first visit to the second input halves: a short
                            # PE fill while their DMA completes (sized for the
                            # slow-DVFS cores that define the max-core time)
                            for _ in range(16):
                                nc.tensor.matmul(warm[:, :], ident[:, :],
                                                 ident[:, :],
                                                 start=True, stop=True)
                        nc.tensor.matmul(
                            ps[:, :],
                            wq_h[dc // HC][:, dc % HC,
                                           oc * P:(oc + 1) * P],
                            xq_chunk(g, dc),
                            start=(dc == 0), stop=(dc == DC - 1))
                    nc.scalar.copy(qt_sb[:, oc, g * 512:(g + 1) * 512],
                                   ps[:, :])

            emit_gproj(0)

            # stream the attention operands while gT group 0 is computed;
            # first-use order: first-half keys + mask for the slot 0-3 chain,
            # its x rows, then group-1 queries and the second-half keys
            qn = NT // 4
            nc.sync.dma_start(out=xkv_k[0][:, :, :], in_=xkv_d[:, 0, :, :])
            nc.scalar.dma_start(out=mask_sb[:, :, :], in_=mask_d[:, :, :])
            nc.sync.dma_start(out=xv_q[0][:, :, :], in_=xkvr_d[:, 0:qn, :])
            nc.scalar.dma_start(out=wv_h[0][:, :, :], in_=wv_d[:, 0:HC, :])
            nc.sync.dma_start(out=xv_q[1][:, :, :], in_=xkvr_d[:, qn:2 * qn, :])
            nc.sync.dma_start(out=xq_1[:, :, :], in_=xq_d[:, 1, :, :])
            nc.sync.dma_start(out=xkv_k[1][:, :, :], in_=xkv_d[:, 1, :, :])
            nc.scalar.dma_start(out=wv_h[1][:, :, :], in_=wv_d[:, HC:DC, :])
            for qtr in range(2, 4):
                nc.sync.dma_start(out=xv_q[qtr][:, :, :],
                                  in_=xkvr_d[:, qtr * qn:(qtr + 1) * qn, :])

            # ---- attention: key-major transposed scores, slot-major AV.
            # score tiles are emitted in pieces split at the q=512 group
            # boundary, so part 0 (query slots 0-3) only depends on the first
            # half of the gT projection and can run before group 1 exists.
            def emit_sc(kt, part):
                s0 = kt // 2                    # first (and only masked) slot
                pT = pT_k[kt]
                if kt < NT // 2 and part == 0:
                    c0, c1 = s0 * P, 512        # group-0 columns, has mask
                    masked = True
                elif kt < NT // 2:
                    c0, c1 = 512, QROWS         # group-1 columns
                    masked = False
                else:
                    c0, c1 = s0 * P, QROWS      # entirely inside group 1
                    masked = True
                pw = c1 - c0
                ps = ps_s.tile([P, 512], f32, tag="mm512",
                               name=f"tps{kt}_{part}")
                for dc in range(DC):
                    nc.tensor.matmul(
                        ps[:, :pw], xkv_c(dc, kt), qt_sb[:, dc, c0:c1],
                        start=(dc == 0), stop=(dc == DC - 1))
                if masked:                      # diagonal/pad mask: slot s0
                    nc.vector.tensor_add(ps[:, 0:P], ps[:, 0:P],
                                         mask_sb[:, kt, :])
                nc.scalar.activation(
                    pT[:, c0 - s0 * P:c1 - s0 * P], ps[:, :pw],
                    mybir.ActivationFunctionType.Exp, scale=SCALE)

            slot_bufs = {}

            def emit_av(s):
                L = CAP[s]
                out_ps = ps_o.tile([P, DV], f32, tag="mmout",
                                   name=f"ops{s}")
                for kt in range(L):
                    lhs = pT_k[kt][:, (s - kt // 2) * P:(s - kt // 2 + 1) * P]
                    xv = xv_q[kt // 4][:, kt % 4, :]
                    nc.tensor.matmul(out_ps[:, 0:512], lhs, xv[:, 0:512],
                                     start=(kt == 0), stop=(kt == L - 1))
                    nc.tensor.matmul(out_ps[:, 512:DV], lhs, xv[:, 512:DV],
                                     start=(kt == 0), stop=(kt == L - 1))
                rinv = small.tile([P, 1], f32, tag="rinv", name=f"rinv{s}")
                nc.vector.reciprocal(rinv[:, :], out_ps[:, D:D + 1])
                # out_ps holds Y = probs @ x_kv [q, d]; stage Y/l to SBUF
                # (softmax normalization folded into the drain copies) so it
                # can be transposed to apply Wv: out = (Y/l) @ Wv^T
                y_lo = osb_pool.tile([P, 512], bf16, tag="ylo", name=f"ylo{s}")
                y_hi = osb_pool.tile([P, 256], bf16, tag="yhi", name=f"yhi{s}")
                nc.scalar.activation(y_lo[:, :], out_ps[:, 0:512],
                                     mybir.ActivationFunctionType.Copy,
                                     scale=rinv[:, :])
                nc.vector.tensor_scalar_mul(y_hi[:, :], out_ps[:, 512:D],
                                            rinv[:, :])
                slot_bufs[s] = (y_lo, y_hi)

            def emit_ytr(s):
                y_lo, y_hi = slot_bufs.pop(s)
                ytT = pt_pool.tile([P, D], bf16, tag="ytT", name=f"ytT{s}")
                for kg in range(2):
                    kn = 4 if kg == 0 else 2
                    tp = ps_tr.tile([P, 512], bf16, tag="tr", name=f"ytp{s}")
                    for j in range(kn):
                        dt = kg * 4 + j
                        ysrc = (y_lo[:, dt * P:(dt + 1) * P] if dt < 4 else
                                y_hi[:, (dt - 4) * P:(dt - 3) * P])
                        nc.tensor.transpose(tp[:, j * P:(j + 1) * P],
                                            ysrc, ident[:, :])
                    nc.vector.tensor_copy(
                        ytT[:, kg * 512:kg * 512 + kn * P],
                        tp[:, 0:kn * P])
                slot_bufs[s] = ytT

            def emit_out2(s):
                ytT = slot_bufs.pop(s)
                last = s == NSLOT - 1
                out2_ps = ps_o.tile([P, D], f32, tag="mmout", name=f"o2ps{s}")
                out_sb = osb_pool.tile([P, D], f32, tag="osb", name=f"osb{s}")

                def drain(c0, c1, eng):
                    cp = (nc.scalar.copy if eng == 0 else
                          nc.vector.tensor_copy)
                    cp(out_sb[:, c0:c1], out2_ps[:, c0:c1])
                    nc.sync.dma_start(out=out_d[s * P:(s + 1) * P, c0:c1],
                                      in_=out_sb[:, c0:c1])

                if not last:
                    for dc in range(DC):
                        nc.tensor.matmul(out2_ps[:, 0:512],
                                         ytT[:, dc * P:(dc + 1) * P],
                                         wv_c[dc][:, 0:512],
                                         start=(dc == 0), stop=(dc == DC - 1))
                    for dc in range(DC):
                        nc.tensor.matmul(out2_ps[:, 512:D],
                                         ytT[:, dc * P:(dc + 1) * P],
                                         wv_c[dc][:, 512:D],
                                         start=(dc == 0), stop=(dc == DC - 1))
                    # already normalized; PSUM->SBUF drain split across the
                    # scalar and vector engines, halves DMA'd independently
                    drain(0, 384, 0)
                    drain(384, D, 1)
                else:
                    # last slot: three accumulation groups drained in narrow
                    # strips so the kernel tail is one short copy + DMA
                    for g, (c0, c1) in enumerate([(0, 512), (512, 640),
                                                  (640, 768)]):
                        ps_g = (out2_ps if g == 0 else
                                ps_s.tile([P, 512], f32, tag="mm512",
                                          name=f"o2t{s}_{g}"))
                        o0 = 0 if g == 0 else c0
                        for dc in range(DC):
                            nc.tensor.matmul(ps_g[:, c0 - o0:c1 - o0],
                                             ytT[:, dc * P:(dc + 1) * P],
                                             wv_c[dc][:, c0:c1],
                                             start=(dc == 0),
                                             stop=(dc == DC - 1))
                        if g == 0:
                            nc.scalar.copy(out_sb[:, 0:256], out2_ps[:, 0:256])
                            nc.scalar.dma_start(
                                out=out_d[s * P:(s + 1) * P, 0:256],
                                in_=out_sb[:, 0:256])
                            nc.vector.tensor_copy(out_sb[:, 256:512],
                                                  out2_ps[:, 256:512])
                            nc.sync.dma_start(
                                out=out_d[s * P:(s + 1) * P, 256:512],
                                in_=out_sb[:, 256:512])
                        else:
                            cp = (nc.scalar.copy if g == 1 else
                                  nc.vector.tensor_copy)
                            cp(out_sb[:, c0:c1], ps_g[:, 0:c1 - c0])
                            dma = (nc.scalar.dma_start if g == 1 else
                                   nc.sync.dma_start)
                            dma(out=out_d[s * P:(s + 1) * P, c0:c1],
                                in_=out_sb[:, c0:c1])

            # group-decoupled pipeline: the part-0 score pieces and the whole
            # slot 0-3 chain depend only on gT group 0, so they run while the
            # group-1 inputs stream in; gproj group 1 is emitted mid-body.
            for kt in range(NT // 2):
                emit_sc(kt, 0)                  # slots 0-3 scorelets
            emit_av(0)
            emit_av(1)
            emit_ytr(0)
            emit_av(2)
            emit_ytr(1)
            emit_out2(0)
            emit_av(3)
            emit_ytr(2)
            emit_out2(1)
            emit_gproj(1)                       # q-group 1 arrives during the
            for kt in range(NT // 2):           # slot 0-3 tail
                emit_sc(kt, 1)
                if kt == 2:
                    emit_ytr(3)
                    emit_out2(2)
            emit_sc(8, 1)
            emit_sc(9, 1)
            emit_av(4)
            emit_out2(3)
            for s in range(5, NSLOT):
                emit_sc(2 * s, 1)
                emit_sc(2 * s + 1, 1)
                emit_ytr(s - 1)
                emit_av(s)
                emit_out2(s - 1)
            emit_ytr(NSLOT - 1)
            emit_out2(NSLOT - 1)

    nc.compile()
    return nc


def _pack(matT):
    """[D, W] (transposed operand) -> [P, DC, W] chunk layout, bf16."""
    d, w = matT.shape
    return np.ascontiguousarray(
        matT.reshape(d // P, P, w).transpose(1, 0, 2)).astype(BF16)


def shard_inputs(x, Wq, Wk, Wv):
    x = np.asarray(x, dtype=np.float32)
    # scores = (x_q Wq^T)(x_k Wk^T)^T = x_q (Wq^T Wk) x_k^T: fold the two
    # projection matrices into M on the host; the device projects only x_q
    M = np.asarray(Wq, np.float32).T @ np.asarray(Wk, np.float32)
    mT = _pack(M)                                        # [P, DC(din), dout]
    wvT = _pack(np.asarray(Wv, np.float32).T)
    in_maps = []
    for c in range(N_CORES):
        b, side = divmod(c, 2)
        qtiles = SIDE_A if side == 0 else SIDE_B
        xb = x[b]                                    # [S, D]
        xkvT = _pack(np.ascontiguousarray(xb.T))     # [P, DC, S]
        xkvT = np.ascontiguousarray(                 # [P, 2, DC, S//2]
            xkvT.reshape(P, DC, 2, S // 2).transpose(0, 2, 1, 3))
        xkvR = np.zeros((P, NT, DV), BF16)           # row-major + ones column
        xkvR[:, :, :D] = xb.astype(BF16).reshape(NT, P, D).transpose(1, 0, 2)
        xkvR[:, :, D] = 1.0
        xq = np.concatenate([xb[t * P:(t + 1) * P] for t in qtiles], axis=0)
        xqT = _pack(np.ascontiguousarray(xq.T))          # [P, DC, QROWS]
        xqT = np.ascontiguousarray(                      # [P, 2, DC, 512]
            xqT.reshape(P, DC, 2, 512).transpose(0, 2, 1, 3))
        # transposed mask per key tile kt: [128 k, 128 q] for slot kt//2
        mask = np.empty((P, NT, P), np.float32)
        for kt in range(NT):
            t0 = qtiles[kt // 2]
            kidx = kt * P + np.arange(P)[:, None]
            qidx = t0 * P + np.arange(P)[None, :]
            mask[:, kt, :] = np.where(kidx <= qidx, 0.0, -1e30)
        mask = mask.astype(BF16)
        in_maps.append({"xqT": xqT, "xkvT": xkvT, "xkvR": xkvR, "mT": mT,
                        "wvT": wvT, "mask": mask})
    return in_maps


def unshard(results):
    out = np.empty((B, S, D), np.float32)
    for c in range(N_CORES):
        b, side = divmod(c, 2)
        qtiles = SIDE_A if side == 0 else SIDE_B
        oc = results[c]["out"]
        for s, t in enumerate(qtiles):
            out[b, t * P:(t + 1) * P] = oc[s * P:(s + 1) * P]
    return out


def run(inputs, trace=False, trace_cores=None):
    """Run on hardware; returns (output, BassKernelResults)."""
    global _NC
    if _NC is None:
        _NC = build()
    in_maps = shard_inputs(inputs["x"], inputs["Wq"], inputs["Wk"],
                           inputs["Wv"])
    res = run_bass_kernel_spmd(_NC, in_maps, core_ids=list(range(N_CORES)),
                               trace=trace, trace_cores=trace_cores)
    return unshard(res.results), res


def kernel(x, Wq, Wk, Wv):
    out, _ = run({"x": x, "Wq": Wq, "Wk": Wk, "Wv": Wv})
    return out



# revision 63
# speedup vs baseline: 1.0265x; 1.0243x over previous
"""Trainium2 Bass kernel for single-head causal attention.

Problem: x:[4,2048,768], Wq/Wk/Wv:[768,768] (torch-Linear layout, y = x @ W.T),
out = causal_softmax(q k^T / sqrt(768)) @ v, all float32.

Sharding (8 NeuronCores, no collectives):
  - core pair (2b, 2b+1) handles batch b.
  - per batch, the 16 query tiles of 128 rows are split between the pair as
    {0,3,4,7,8,11,12,15} and {1,2,5,6,9,10,13,14}. Sorted by causal length
    those are {1,4,5,8,9,12,13,16} and {2,3,6,7,10,11,14,15} key-tiles, so
    both sides fit the same static per-slot key budget {2,4,...,16}: the one
    SPMD graph processes 8 query tiles whose key ranges are padded by at most
    one 128-tile (+6% flops) and the pad/diagonal is handled by a host-
    provided additive mask over the last two key blocks of every slot.
  - scores are computed as (x_q @ M) @ x_k^T with M = Wq^T @ Wk folded on the
    host, so the device never projects K: the "g" projection x_q @ M replaces
    the Q projection at identical cost and the raw transposed x serves as the
    key operand directly.
  - scores are computed TRANSPOSED (key-major): scoresT[k, q] tiles come out
    of the PE in exactly the layout the probs @ x matmul needs as stationary
    operand, so the 128x128 probability transposes disappear. The softmax
    denominator is obtained by appending a ones-column to the x rows used in
    the AV matmul (out_ps[:, 768] = sum_k p_k).
  - host pre-transposes inputs (x^T, M, W^T), packs them into the SBUF chunk
    layout [128, chunk, width], and converts to bf16, so the device does no
    operand transposes; only the per-query-tile Y = probs @ x results are
    transposed on the TensorEngine to apply Wv.
"""

import math
import os
import sys

import numpy as np

if not any(os.path.isdir(os.path.join(p, "concourse")) for p in sys.path):
    sys.path.insert(0, "/opt/trn_rl_repo")

import concourse.bass as bass  # noqa: E402
import concourse.mybir as mybir  # noqa: E402
from concourse import bacc, tile  # noqa: E402
from concourse.bass_utils import run_bass_kernel_spmd  # noqa: E402
from concourse.masks import make_identity  # noqa: E402

import ml_dtypes  # noqa: E402

B, S, D = 4, 2048, 768
P = 128
NT = S // P          # 16 key tiles per batch
DC = D // P          # 6 contraction chunks
NSLOT = 8            # query tiles per core
QROWS = NSLOT * P    # 1024 query rows per core
N_CORES = 8
SCALE = 1.0 / math.sqrt(D)

SIDE_A = [0, 3, 4, 7, 8, 11, 12, 15]   # causal lengths 1,4,5,8,9,12,13,16
SIDE_B = [1, 2, 5, 6, 9, 10, 13, 14]   # causal lengths 2,3,6,7,10,11,14,15
CAP = [2, 4, 6, 8, 10, 12, 14, 16]     # static key tiles per slot (>= real)
DV = D + 4                             # x rows padded with a ones column

BF16 = ml_dtypes.bfloat16

_NC = None


def build():
    """Build + compile the single SPMD graph run by all 8 cores."""
    f32 = mybir.dt.float32
    bf16 = mybir.dt.bfloat16

    nc = bacc.Bacc("TRN2", target_bir_lowering=False, debug=False,
                   num_devices=N_CORES)

    # inputs come pre-packed as [P, chunk, width] (host layout transform)
    xq_d = nc.dram_tensor("xqT", [P, 2, DC, 512], bf16,
                          kind="ExternalInput").ap()
    xkv_d = nc.dram_tensor("xkvT", [P, 2, DC, S // 2], bf16,
                           kind="ExternalInput").ap()
    xkvr_d = nc.dram_tensor("xkvR", [P, NT, DV], bf16,
                            kind="ExternalInput").ap()
    wq_d = nc.dram_tensor("mT", [P, DC, D], bf16,
                          kind="ExternalInput").ap()
    wv_d = nc.dram_tensor("wvT", [P, DC, D], bf16, kind="ExternalInput").ap()
    mask_d = nc.dram_tensor("mask", [P, NT, P], bf16,
                            kind="ExternalInput").ap()
    out_d = nc.dram_tensor("out", [QROWS, D], f32, kind="ExternalOutput").ap()

    with tile.TileContext(nc) as tc:
        with (
            tc.tile_pool(name="const", bufs=1) as const,
            tc.tile_pool(name="pt", bufs=3) as pt_pool,
            tc.tile_pool(name="osb", bufs=2) as osb_pool,
            tc.tile_pool(name="small", bufs=2) as small,
            tc.tile_pool(name="ps_s", bufs=2, space="PSUM") as ps_s,
            tc.tile_pool(name="ps_tr", bufs=2, space="PSUM") as ps_tr,
            tc.tile_pool(name="ps_o", bufs=2, space="PSUM") as ps_o,
        ):
            # ---- persistent SBUF tensors, split in two halves of 3 d-chunks
            # each so input DMA (12KB+ descriptors) overlaps the projections
            HC = DC // 2
            wq_h = [const.tile([P, HC, D], bf16, tag=f"wqh{h}", name=f"wqh{h}")
                    for h in range(2)]
            wv_h = [const.tile([P, HC, D], bf16, tag=f"wvh{h}", name=f"wvh{h}")
                    for h in range(2)]
            # group 0 in two half-tiles so its first matmuls only wait on the
            # first half's DMA; group 1 arrives while group 0 is computed
            xq_0 = [const.tile([P, HC, 512], bf16, tag=f"xq0{h}",
                               name=f"xq0{h}") for h in range(2)]
            xq_1 = const.tile([P, DC, 512], bf16, tag="xq1")

            def xq_chunk(g, dc):
                if g == 0:
                    return xq_0[dc // HC][:, dc % HC, :]
                return xq_1[:, dc, :]
            # keys split by k-range so scores can start after half the DMA
            xkv_k = [const.tile([P, DC, S // 2], bf16, tag=f"xkvk{h}",
                                name=f"xkvk{h}") for h in range(2)]

            def chunk(tiles, dc):
                return tiles[dc // HC][:, dc % HC, :]

            wv_c = [chunk(wv_h, c) for c in range(DC)]

            def xkv_c(dc, kt):
                # [128 d, 128 k] block of x^T for key tile kt, d-chunk dc
                j, kk = divmod(kt, NT // 2)
                return xkv_k[j][:, dc, kk * P:(kk + 1) * P]
            mask_sb = const.tile([P, NT, P], bf16, tag="mask")
            ident = const.tile([P, P], bf16, tag="ident")
            qt_sb = const.tile([P, DC, QROWS], bf16, tag="qt")
            xv_q = [const.tile([P, NT // 4, DV], bf16, tag=f"xvq{h}",
                               name=f"xvq{h}") for h in range(4)]
            # persistent transposed-probability tiles, one per key tile kt:
            # [128 k, (NSLOT - kt//2)*128 q] covering slots kt//2 .. 7
            pT_k = [const.tile([P, (NSLOT - kt // 2) * P], bf16,
                               tag=f"pTk{kt}", name=f"pTk{kt}")
                    for kt in range(NT)]

            # priority-ordered input DMAs, descriptor issue split between the
            # two HWDGE engines (Sync: x tensors, Scalar: weights/mask) so
            # posting isn't serialized on Sync at ~0.6us per dma_start.
            # xq group 0 and M stream in d-chunk halves so the projection can
            # begin accumulating as soon as the first halves land.
            nc.sync.dma_start(out=xq_0[0][:, :, :], in_=xq_d[:, 0, 0:HC, :])
            nc.scalar.dma_start(out=wq_h[0][:, :, :], in_=wq_d[:, 0:HC, :])
            nc.sync.dma_start(out=xq_0[1][:, :, :], in_=xq_d[:, 0, HC:DC, :])
            nc.scalar.dma_start(out=wq_h[1][:, :, :], in_=wq_d[:, HC:DC, :])
            make_identity(nc, ident[:, :])

            # HAM warm-up: keep the PE busy (and the clock ramping) while the
            # first input halves stream in
            warm = ps_tr.tile([P, P], f32, tag="tr", name="warm")
            for _ in range(36):
                nc.tensor.matmul(warm[:, :], ident[:, :], ident[:, :],
                                 start=True, stop=True)

            # ---- gT[dout,q] = (x_q @ M)^T projection (group-major, streamed
            # under the input DMA; M = Wq^T Wk folded on host)
            def emit_gproj(g):
                for oc in range(DC):
                    ps = ps_s.tile([P, 512], f32, tag="mm512")
                    for dc in range(DC):
                        nc.tensor.matmul(
                            ps[:, :],
                            wq_h[dc // HC][:, dc % HC,
                                           oc * P:(oc + 1) * P],
                            xq_chunk(g, dc),
                            start=(dc == 0), stop=(dc == DC - 1))
                    nc.scalar.copy(qt_sb[:, oc, g * 512:(g + 1) * 512],
                                   ps[:, :])

            def emit_gproj0():
                # group 0 streamed at half-input granularity: the first-half
                # accumulations of FOUR output chunks run on the early DMA
                # halves (2 ps_s + 2 borrowed mmout banks as accumulators),
                # so the PE has real work while the second halves land
                accs = []
                for oc in range(4):
                    ps = (ps_s.tile([P, 512], f32, tag="mm512",
                                    name=f"gpa{oc}") if oc < 2 else
                          ps_o.tile([P, DV], f32, tag="mmout",
                                    name=f"gpa{oc}"))
                    for dc in range(HC):
                        nc.tensor.matmul(
                            ps[:, 0:512],
                            wq_h[0][:, dc, oc * P:(oc + 1) * P],
                            xq_chunk(0, dc),
                            start=(dc == 0), stop=False)
                    accs.append(ps)
                # short PE fill while the second halves' DMA completes (sized
                # for the slow-DVFS cores that define the max-core time)
                for _ in range(16):
                    nc.tensor.matmul(warm[:, :], ident[:, :], ident[:, :],
                                     start=True, stop=True)
                for oc in range(4):
                    ps = accs[oc]
                    for dc in range(HC, DC):
                        nc.tensor.matmul(
                            ps[:, 0:512],
                            wq_h[1][:, dc - HC, oc * P:(oc + 1) * P],
                            xq_chunk(0, dc),
                            start=False, stop=(dc == DC - 1))
                    nc.scalar.copy(qt_sb[:, oc, 0:512], ps[:, 0:512])
                for oc in range(4, DC):
                    ps = ps_s.tile([P, 512], f32, tag="mm512",
                                   name=f"gpb{oc}")
                    for dc in range(DC):
                        nc.tensor.matmul(
                            ps[:, :],
                            wq_h[dc // HC][:, dc % HC,
                                           oc * P:(oc + 1) * P],
                            xq_chunk(0, dc),
                            start=(dc == 0), stop=(dc == DC - 1))
                    nc.scalar.copy(qt_sb[:, oc, 0:512], ps[:, :])

            emit_gproj0()

            # stream the attention operands while gT group 0 is computed;
            # first-use order: first-half keys + mask for the slot 0-3 chain,
            # its x rows, then group-1 queries and the second-half keys
            qn = NT // 4
            nc.sync.dma_start(out=xkv_k[0][:, :, :], in_=xkv_d[:, 0, :, :])
            nc.scalar.dma_start(out=mask_sb[:, :, :], in_=mask_d[:, :, :])
            nc.sync.dma_start(out=xv_q[0][:, :, :], in_=xkvr_d[:, 0:qn, :])
            nc.scalar.dma_start(out=wv_h[0][:, :, :], in_=wv_d[:, 0:HC, :])
            nc.sync.dma_start(out=xv_q[1][:, :, :], in_=xkvr_d[:, qn:2 * qn, :])
            nc.sync.dma_start(out=xq_1[:, :, :], in_=xq_d[:, 1, :, :])
            nc.sync.dma_start(out=xkv_k[1][:, :, :], in_=xkv_d[:, 1, :, :])
            nc.scalar.dma_start(out=wv_h[1][:, :, :], in_=wv_d[:, HC:DC, :])
            for qtr in range(2, 4):
                nc.sync.dma_start(out=xv_q[qtr][:, :, :],
                                  in_=xkvr_d[:, qtr * qn:(qtr + 1) * qn, :])

            # ---- attention: key-major transposed scores, slot-major AV.
            # score tiles are emitted in pieces split at the q=512 group
            # boundary, so part 0 (query slots 0-3) only depends on the first
            # half of the gT projection and can run before group 1 exists.
            def emit_sc(kt, part):
                s0 = kt // 2                    # first (and only masked) slot
                pT = pT_k[kt]
                if kt < NT // 2 and part == 0:
                    c0, c1 = s0 * P, 512        # group-0 columns, has mask
                    masked = True
                elif kt < NT // 2:
                    c0, c1 = 512, QROWS         # group-1 columns
                    masked = False
                else:
                    c0, c1 = s0 * P, QROWS      # entirely inside group 1
                    masked = True
                pw = c1 - c0
                ps = ps_s.tile([P, 512], f32, tag="mm512",
                               name=f"tps{kt}_{part}")
                for dc in range(DC):
                    nc.tensor.matmul(
                        ps[:, :pw], xkv_c(dc, kt), qt_sb[:, dc, c0:c1],
                        start=(dc == 0), stop=(dc == DC - 1))
                if masked:                      # diagonal/pad mask: slot s0
                    nc.vector.tensor_add(ps[:, 0:P], ps[:, 0:P],
                                         mask_sb[:, kt, :])
                nc.scalar.activation(
                    pT[:, c0 - s0 * P:c1 - s0 * P], ps[:, :pw],
                    mybir.ActivationFunctionType.Exp, scale=SCALE)

            slot_bufs = {}

            def emit_av(s):
                L = CAP[s]
                out_ps = ps_o.tile([P, DV], f32, tag="mmout",
                                   name=f"ops{s}")
                for kt in range(L):
                    lhs = pT_k[kt][:, (s - kt // 2) * P:(s - kt // 2 + 1) * P]
                    xv = xv_q[kt // 4][:, kt % 4, :]
                    nc.tensor.matmul(out_ps[:, 0:512], lhs, xv[:, 0:512],
                                     start=(kt == 0), stop=(kt == L - 1))
                    nc.tensor.matmul(out_ps[:, 512:DV], lhs, xv[:, 512:DV],
                                     start=(kt == 0), stop=(kt == L - 1))
                rinv = small.tile([P, 1], f32, tag="rinv", name=f"rinv{s}")
                nc.vector.reciprocal(rinv[:, :], out_ps[:, D:D + 1])
                # out_ps holds Y = probs @ x_kv [q, d]; stage Y/l to SBUF
                # (softmax normalization folded into the drain copies) so it
                # can be transposed to apply Wv: out = (Y/l) @ Wv^T
                y_lo = osb_pool.tile([P, 512], bf16, tag="ylo", name=f"ylo{s}")
                y_hi = osb_pool.tile([P, 256], bf16, tag="yhi", name=f"yhi{s}")
                nc.scalar.activation(y_lo[:, :], out_ps[:, 0:512],
                                     mybir.ActivationFunctionType.Copy,
                                     scale=rinv[:, :])
                nc.vector.tensor_scalar_mul(y_hi[:, :], out_ps[:, 512:D],
                                            rinv[:, :])
                slot_bufs[s] = (y_lo, y_hi)

            def emit_ytr(s):
                y_lo, y_hi = slot_bufs.pop(s)
                ytT = pt_pool.tile([P, D], bf16, tag="ytT", name=f"ytT{s}")
                for kg in range(2):
                    kn = 4 if kg == 0 else 2
                    tp = ps_tr.tile([P, 512], bf16, tag="tr", name=f"ytp{s}")
                    for j in range(kn):
                        dt = kg * 4 + j
                        ysrc = (y_lo[:, dt * P:(dt + 1) * P] if dt < 4 else
                                y_hi[:, (dt - 4) * P:(dt - 3) * P])
                        nc.tensor.transpose(tp[:, j * P:(j + 1) * P],
                                            ysrc, ident[:, :])
                    nc.vector.tensor_copy(
                        ytT[:, kg * 512:kg * 512 + kn * P],
                        tp[:, 0:kn * P])
                slot_bufs[s] = ytT

            def emit_out2(s):
                ytT = slot_bufs.pop(s)
                last = s == NSLOT - 1
                out2_ps = ps_o.tile([P, D], f32, tag="mmout", name=f"o2ps{s}")
                out_sb = osb_pool.tile([P, D], f32, tag="osb", name=f"osb{s}")

                def drain(c0, c1, eng):
                    cp = (nc.scalar.copy if eng == 0 else
                          nc.vector.tensor_copy)
                    cp(out_sb[:, c0:c1], out2_ps[:, c0:c1])
                    nc.sync.dma_start(out=out_d[s * P:(s + 1) * P, c0:c1],
                                      in_=out_sb[:, c0:c1])

                if not last:
                    for dc in range(DC):
                        nc.tensor.matmul(out2_ps[:, 0:512],
                                         ytT[:, dc * P:(dc + 1) * P],
                                         wv_c[dc][:, 0:512],
                                         start=(dc == 0), stop=(dc == DC - 1))
                    for dc in range(DC):
                        nc.tensor.matmul(out2_ps[:, 512:D],
                                         ytT[:, dc * P:(dc + 1) * P],
                                         wv_c[dc][:, 512:D],
                                         start=(dc == 0), stop=(dc == DC - 1))
                    # already normalized; PSUM->SBUF drain split across the
                    # scalar and vector engines, halves DMA'd independently
                    drain(0, 384, 0)
                    drain(384, D, 1)
                else:
                    # last slot: three accumulation groups drained in narrow
                    # strips so the kernel tail is one short copy + DMA
                    for g, (c0, c1) in enumerate([(0, 512), (512, 640),
                                                  (640, 768)]):
                        ps_g = (out2_ps if g == 0 else
                                ps_s.tile([P, 512], f32, tag="mm512",
                                          name=f"o2t{s}_{g}"))
                        o0 = 0 if g == 0 else c0
                        for dc in range(DC):
                            nc.tensor.matmul(ps_g[:, c0 - o0:c1 - o0],
                                             ytT[:, dc * P:(dc + 1) * P],
                                             wv_c[dc][:, c0:c1],
                                             start=(dc == 0),
                                             stop=(dc == DC - 1))
                        if g == 0:
                            nc.scalar.copy(out_sb[:, 0:256], out2_ps[:, 0:256])
                            nc.scalar.dma_start(
                                out=out_d[s * P:(s + 1) * P, 0:256],
                                in_=out_sb[:, 0:256])
                            nc.vector.tensor_copy(out_sb[:, 256:512],
                                                  out2_ps[:, 256:512])
                            nc.sync.dma_start(
                                out=out_d[s * P:(s + 1) * P, 256:512],
                                in_=out_sb[:, 256:512])
                        else:
                            cp = (nc.scalar.copy if g == 1 else
                                  nc.vector.tensor_copy)
                            cp(out_sb[:, c0:c1], ps_g[:, 0:c1 - c0])
                            dma = (nc.scalar.dma_start if g == 1 else
                                   nc.sync.dma_start)
                            dma(out=out_d[s * P:(s + 1) * P, c0:c1],
                                in_=out_sb[:, c0:c1])

            # group-decoupled pipeline: the part-0 score pieces and the whole
            # slot 0-3 chain depend only on gT group 0, so they run while the
            # group-1 inputs stream in; gproj group 1 is emitted mid-body.
            for kt in range(NT // 2):
                emit_sc(kt, 0)                  # slots 0-3 scorelets
            emit_av(0)
            emit_av(1)
            emit_ytr(0)
            emit_av(2)
            emit_ytr(1)
            emit_out2(0)
            emit_av(3)
            emit_ytr(2)
            emit_out2(1)
            emit_gproj(1)                       # q-group 1 arrives during the
            for kt in range(NT // 2):           # slot 0-3 tail
                emit_sc(kt, 1)
                if kt == 2:
                    emit_ytr(3)
                    emit_out2(2)
            emit_sc(8, 1)
            emit_sc(9, 1)
            emit_av(4)
            emit_out2(3)
            for s in range(5, NSLOT):
                emit_sc(2 * s, 1)
                emit_sc(2 * s + 1, 1)
                emit_ytr(s - 1)
                emit_av(s)
                emit_out2(s - 1)
            emit_ytr(NSLOT - 1)
            emit_out2(NSLOT - 1)

    nc.compile()
    return nc


def _pack(matT):
    """[D, W] (transposed operand) -> [P, DC, W] chunk layout, bf16."""
    d, w = matT.shape
    return np.ascontiguousarray(
        matT.reshape(d // P, P, w).transpose(1, 0, 2)).astype(BF16)


def shard_inputs(x, Wq, Wk, Wv):
    x = np.asarray(x, dtype=np.float32)
    # scores = (x_q Wq^T)(x_k Wk^T)^T = x_q (Wq^T Wk) x_k^T: fold the two
    # projection matrices into M on the host; the device projects only x_q
    M = np.asarray(Wq, np.float32).T @ np.asarray(Wk, np.float32)
    mT = _pack(M)                                        # [P, DC(din), dout]
    wvT = _pack(np.asarray(Wv, np.float32).T)
    in_maps = []
    for c in range(N_CORES):
        b, side = divmod(c, 2)
        qtiles = SIDE_A if side == 0 else SIDE_B
        xb = x[b]                                    # [S, D]
        xkvT = _pack(np.ascontiguousarray(xb.T))     # [P, DC, S]
        xkvT = np.ascontiguousarray(                 # [P, 2, DC, S//2]
            xkvT.reshape(P, DC, 2, S // 2).transpose(0, 2, 1, 3))
        xkvR = np.zeros((P, NT, DV), BF16)           # row-major + ones column
        xkvR[:, :, :D] = xb.astype(BF16).reshape(NT, P, D).transpose(1, 0, 2)
        xkvR[:, :, D] = 1.0
        xq = np.concatenate([xb[t * P:(t + 1) * P] for t in qtiles], axis=0)
        xqT = _pack(np.ascontiguousarray(xq.T))          # [P, DC, QROWS]
        xqT = np.ascontiguousarray(                      # [P, 2, DC, 512]
            xqT.reshape(P, DC, 2, 512).transpose(0, 2, 1, 3))
        # transposed mask per key tile kt: [128 k, 128 q] for slot kt//2
        mask = np.empty((P, NT, P), np.float32)
        for kt in range(NT):
            t0 = qtiles[kt // 2]
            kidx = kt * P + np.arange(P)[:, None]
            qidx = t0 * P + np.arange(P)[None, :]
            mask[:, kt, :] = np.where(kidx <= qidx, 0.0, -1e30)
        mask = mask.astype(BF16)
        in_maps.append({"xqT": xqT, "xkvT": xkvT, "xkvR": xkvR, "mT": mT,
                        "wvT": wvT, "mask": mask})
    return in_maps


def unshard(results):
    out = np.empty((B, S, D), np.float32)
    for c in range(N_CORES):
        b, side = divmod(c, 2)
        qtiles = SIDE_A if side == 0 else SIDE_B
        oc = results[c]["out"]
        for s, t in enumerate(qtiles):
            out[b, t * P:(t + 1) * P] = oc[s * P:(s + 1) * P]
    return out


def run(inputs, trace=False, trace_cores=None):
    """Run on hardware; returns (output, BassKernelResults)."""
    global _NC
    if _NC is None:
        _NC = build()
    in_maps = shard_inputs(inputs["x"], inputs["Wq"], inputs["Wk"],
                           inputs["Wv"])
    res = run_bass_kernel_spmd(_NC, in_maps, core_ids=list(range(N_CORES)),
                               trace=trace, trace_cores=trace_cores)
    return unshard(res.results), res


def kernel(x, Wq, Wk, Wv):
    out, _ = run({"x": x, "Wq": Wq, "Wk": Wk, "Wv": Wv})
    return out

